# revision 1
# baseline (speedup 1.0000x reference)
"""Trainium2 Bass kernel for AttnBlock (GroupNorm + single-head spatial
self-attention + projection + residual).

Sharding: 8 cores = 4 batches x 2 query-halves. Each core computes
GN + K/V for its batch (duplicated within the pair) and attention +
projection for its half of the 4096 query positions. No collectives.

Math per core (batch b, query half h, N=4096 keys, NQ=2048 queries):
  h_ = groupnorm(x[b])                    [C, N]
  K  = WkT.T @ h_ + bk                    [C, N]
  VT = (h_.T @ WvT)                       [N, C]   (no bv; folded into bpp)
  Q  = WqT.T @ gn(xq) + bq                [C, NQ]
  S^T = K.T @ Q * C^-0.5  -> E = exp(S^T) [N, NQ]  (no max-sub; scores are O(5))
  O0 = VT.T @ E (unnormalized)            [C, NQ];  den = sum_j E
  out = xq + (WpT.T @ O0) / den + bpp     where bpp = Wp@bv + bp
"""
import math
import numpy as np

import bass_rust

import concourse.bass as bass
import concourse.bacc as bacc
import concourse.tile as tile
from concourse import mybir
from concourse.bass_utils import run_bass_kernel_spmd

F32 = mybir.dt.float32
F32R = mybir.dt.float32r
AF = mybir.ActivationFunctionType
ALU = mybir.AluOpType

C = 512          # channels
N = 4096         # spatial positions (keys)
NQ = 2048        # queries per core
CT = 4           # channel tiles of 128
ICN = 4          # i-chunks per core
ICW = 512        # i-chunk width
JBN = 16         # j-blocks (128 wide) per half
JCN = 4          # j 512-chunks per half
GROUPS = 32
EPS = 1e-6
INV = 1.0 / math.sqrt(C)
BN_FMAX = 512


def _emit(nc, tc, ctx, tens, rep):
    r = f"r{rep}_"
    XF, XQ = tens["XF"], tens["XQ"]
    WQT, WKT, WVT, WPT = tens["WQT"], tens["WKT"], tens["WVT"], tens["WPT"]
    GM = tens["GM"]
    OUT = tens["OUT"]

    const = ctx.enter_context(tc.tile_pool(name=r + "const", bufs=1))
    pw2 = ctx.enter_context(tc.tile_pool(name=r + "pw2", bufs=1))
    kpool = ctx.enter_context(tc.tile_pool(name=r + "kres", bufs=1))
    vpool = ctx.enter_context(tc.tile_pool(name=r + "vres", bufs=1))
    dramp = ctx.enter_context(tc.tile_pool(name=r + "dram", bufs=1, space="DRAM"))
    pps = ctx.enter_context(tc.tile_pool(name=r + "pps", bufs=4, space="PSUM"))
    ph = tc.alloc_tile_pool(name=r + "ph", bufs=1)
    x_t = [ph.tile([128, N], F32R, name=f"{r}x{t}", tag=f"x{t}") for t in range(CT)]
    for t in range(CT):
        for xc in range(8):
            nc.sync.dma_start(
                out=x_t[t][:, xc * 512:(xc + 1) * 512],
                in_=XF[t * 128:(t + 1) * 128, xc * 512:(xc + 1) * 512].bitcast(F32R))

    # ---- constants: gm first, then one packed vector DMA (issue-order
    # ---- matters: everything on the sync queue sits behind the x load)
    gm_t = const.tile([128, 128], F32, name=r + "gm")
    nc.sync.dma_start(out=gm_t, in_=GM[:, :])
    cvec = const.tile([128, 20], F32, name=r + "cvec")
    nc.sync.dma_start(out=cvec, in_=tens["CVEC"][:, :])
    bq_t = [cvec[:, cb:cb + 1] for cb in range(CT)]
    bk_t = [cvec[:, 4 + cb:5 + cb] for cb in range(CT)]
    bpp_t = [cvec[:, 8 + cb:9 + cb] for cb in range(CT)]
    gns_t = [cvec[:, 12 + t:13 + t] for t in range(CT)]
    gnb_t = [cvec[:, 16 + t:17 + t] for t in range(CT)]
    eps_t = const.tile([128, 1], F32, name=r + "eps")
    nc.vector.memset(eps_t, EPS)
    a_t = [const.tile([128, 1], F32, name=f"{r}a{t}", tag=f"a{t}") for t in range(CT)]
    c2_t = [const.tile([128, 1], F32, name=f"{r}c2{t}", tag=f"c2{t}") for t in range(CT)]
    xr0 = [const.tile([128, ICW], F32, name=f"{r}xr0_{t}", tag=f"xr0{t}")
           for t in range(CT)]
    for t in range(CT):
        nc.sync.dma_start(out=xr0[t], in_=XQ[t * 128:(t + 1) * 128, 0:ICW])
    xn0 = [const.tile([128, ICW], F32R, name=f"{r}xn0_{t}", tag=f"xn0{t}")
           for t in range(CT)]

    wq_t = [pw2.tile([128, C], F32R, name=f"{r}wq{t}", tag=f"wq{t}") for t in range(CT)]
    wp_t = [pw2.tile([128, C], F32R, name=f"{r}wp{t}", tag=f"wp{t}") for t in range(CT)]

    # resident K (one j-half) and VT (one j-half) tiles
    k_sb = [[kpool.tile([128, 512], F32R, name=f"{r}k{cb}_{jc}", tag=f"k{cb}_{jc}")
             for jc in range(JCN)] for cb in range(CT)]
    vt_sb = [vpool.tile([128, 512], F32R, name=f"{r}vt{jb}", tag=f"vt{jb}")
             for jb in range(JBN)]

    # DRAM scratch
    k1d = [[dramp.tile([128, 512], F32, name=f"{r}k1d{cb}_{jc}", tag=f"k1d{cb}_{jc}")
            for jc in range(JCN)] for cb in range(CT)]
    vt1d = [dramp.tile([128, 512], F32, name=f"{r}vt1d{jb}", tag=f"vt1d{jb}")
            for jb in range(JBN)]

    # ================= PHASE 1: GN, VT, K =================
    with (
        tc.tile_pool(name=r + "pw1", bufs=1) as pw1,
        tc.tile_pool(name=r + "pgn", bufs=2) as pgn,
        tc.tile_pool(name=r + "stage", bufs=4) as stage,
        tc.tile_pool(name=r + "gps", bufs=2, space="PSUM") as gps,
    ):
        wk_t = [pw1.tile([128, C], F32R, name=f"{r}wk{t}", tag=f"wk{t}") for t in range(CT)]
        wv_t = [pw1.tile([128, C], F32R, name=f"{r}wv{t}", tag=f"wv{t}") for t in range(CT)]
        for t in range(CT):
            nc.sync.dma_start(out=wv_t[t], in_=WVT[t].bitcast(F32R))
        for t in range(CT):
            nc.sync.dma_start(out=wk_t[t], in_=WKT[t].bitcast(F32R))
        for t in range(CT):
            nc.sync.dma_start(out=wq_t[t], in_=WQT[t].bitcast(F32R))
        for t in range(CT):
            nc.sync.dma_start(out=wp_t[t], in_=WPT[t].bitcast(F32R))


        # --- per-channel stats -> group stats (via indicator matmul) ---
        for t in range(CT):
            xv = x_t[t].bitcast(F32)
            stats = pgn.tile([128, N // BN_FMAX, 6], F32, name=f"{r}st{t}", tag="stats")
            for s in range(N // BN_FMAX):
                nc.vector.bn_stats(out=stats[:, s, :], in_=xv[:, s * BN_FMAX:(s + 1) * BN_FMAX])
            mv = pgn.tile([128, 2], F32, name=f"{r}mv{t}", tag="mv")
            nc.vector.bn_aggr(out=mv, in_=stats)
            # t2 = [mean, var + mean^2]
            t2 = pgn.tile([128, 2], F32, name=f"{r}t2{t}", tag="t2")
            nc.vector.tensor_copy(out=t2[:, 0:1], in_=mv[:, 0:1])
            sq = pgn.tile([128, 1], F32, name=f"{r}sq{t}", tag="sq")
            nc.vector.tensor_mul(out=sq, in0=mv[:, 0:1], in1=mv[:, 0:1])
            nc.vector.tensor_add(out=t2[:, 1:2], in0=mv[:, 1:2], in1=sq)
            chp = gps.tile([128, 2], F32, name=f"{r}chp{t}", tag="gp")
            nc.tensor.matmul(chp, gm_t, t2, start=True, stop=True)
            ch = pgn.tile([128, 2], F32, name=f"{r}ch{t}", tag="ch")
            nc.vector.tensor_copy(out=ch, in_=chp)
            gmean, gmsq = ch[:, 0:1], ch[:, 1:2]
            sg = pgn.tile([128, 1], F32, name=f"{r}sg{t}", tag="sg")
            nc.vector.tensor_mul(out=sg, in0=gmean, in1=gmean)
            gv = pgn.tile([128, 1], F32, name=f"{r}gv{t}", tag="gv")
            nc.vector.tensor_sub(out=gv, in0=gmsq, in1=sg)
            nc.scalar.activation(out=gv, in_=gv, func=AF.Sqrt, bias=eps_t, scale=1.0)
            nc.vector.reciprocal(out=gv, in_=gv)
            nc.vector.tensor_mul(out=a_t[t], in0=gv, in1=gns_t[t])
            tmp = pgn.tile([128, 1], F32, name=f"{r}tm{t}", tag="tm")
            nc.vector.tensor_mul(out=tmp, in0=gmean, in1=a_t[t])
            nc.vector.tensor_sub(out=c2_t[t], in0=gnb_t[t], in1=tmp)

        # --- h = x * a + c2 (in place, rounded to f32r) ---
        H2 = N // 2
        for t in range(CT):
            for hh in range(2):
                sl = slice(hh * H2, (hh + 1) * H2)
                job = 2 * t + hh
                if job % 8 in (0, 3, 6):
                    nc.scalar.activation(
                        out=x_t[t][:, sl], in_=x_t[t][:, sl].bitcast(F32),
                        func=AF.Identity, bias=c2_t[t], scale=a_t[t])
                elif job % 8 in (1, 4, 7):
                    nc.vector.tensor_scalar(
                        out=x_t[t][:, sl], in0=x_t[t][:, sl].bitcast(F32),
                        scalar1=a_t[t], scalar2=c2_t[t], op0=ALU.mult, op1=ALU.add)
                else:
                    nc.gpsimd.tensor_scalar(
                        out=x_t[t][:, sl], in0=x_t[t][:, sl].bitcast(F32),
                        scalar1=a_t[t], scalar2=c2_t[t], op0=ALU.mult, op1=ALU.add)

        for t in range(CT):
            nc.vector.tensor_scalar(out=xn0[t], in0=xr0[t], scalar1=a_t[t],
                                    scalar2=c2_t[t], op0=ALU.mult, op1=ALU.add)

        # --- VT = h.T @ WvT : [N, C]; first half resident, second spilled ---
        for jb in range(2 * JBN):
            vp = pps.tile([128, 512], F32, name=f"{r}vp{jb}", tag="mm")
            for t in range(CT):
                nc.tensor.matmul(vp, x_t[t][:, jb * 128:(jb + 1) * 128], wv_t[t],
                                 start=(t == 0), stop=(t == CT - 1))
            if jb < JBN:
                nc.scalar.copy(out=vt_sb[jb], in_=vp)
            else:
                vs = stage.tile([128, 512], F32R, name=f"{r}vs{jb}", tag="vs")
                nc.scalar.copy(out=vs, in_=vp)
                nc.sync.dma_start(out=vt1d[jb - JBN], in_=vs.bitcast(F32))

        # --- K = WkT.T @ h + bk : [C, N]; first j-half resident, rest spilled ---
        for cb in range(CT):
            for jc in range(2 * JCN):
                kp = pps.tile([128, 512], F32, name=f"{r}kp{cb}_{jc}", tag="mm")
                for t in range(CT):
                    nc.tensor.matmul(kp, wk_t[t][:, cb * 128:(cb + 1) * 128],
                                     x_t[t][:, jc * 512:(jc + 1) * 512],
                                     start=(t == 0), stop=(t == CT - 1))
                if jc < JCN:
                    nc.scalar.add(out=k_sb[cb][jc], in_=kp, add=bk_t[cb])
                else:
                    ks = stage.tile([128, 512], F32R, name=f"{r}ks{cb}_{jc}", tag="ks")
                    nc.scalar.add(out=ks, in_=kp, add=bk_t[cb])
                    nc.sync.dma_start(out=k1d[cb][jc - JCN], in_=ks.bitcast(F32))

    # ================= PHASE 2a: Q =================
    ph.release()
    qpool = ctx.enter_context(tc.tile_pool(name=r + "qres", bufs=1))
    q_sb = [qpool.tile([128, NQ], F32R, name=f"{r}q{t}", tag=f"q{t}") for t in range(CT)]
    with (
        tc.tile_pool(name=r + "p2a", bufs=1) as p2a,
        tc.tile_pool(name=r + "qps", bufs=2, space="PSUM") as qps,
    ):
        for ic in range(ICN):
            if ic == 0:
                xqn = xn0
            else:
                xqn = []
                for t in range(CT):
                    xr = p2a.tile([128, ICW], F32, name=f"{r}xr{t}", tag=f"xr{t}")
                    nc.sync.dma_start(
                        out=xr,
                        in_=XQ[t * 128:(t + 1) * 128, ic * ICW:(ic + 1) * ICW])
                    xn = p2a.tile([128, ICW], F32R, name=f"{r}xn{t}", tag=f"xn{t}")
                    nc.vector.tensor_scalar(out=xn, in0=xr, scalar1=a_t[t],
                                            scalar2=c2_t[t], op0=ALU.mult,
                                            op1=ALU.add)
                    xqn.append(xn)
            for cb in range(CT):
                qp = qps.tile([128, ICW], F32, name=f"{r}qp{cb}", tag="qp")
                for t in range(CT):
                    nc.tensor.matmul(qp, wq_t[t][:, cb * 128:(cb + 1) * 128], xqn[t],
                                     start=(t == 0), stop=(t == CT - 1))
                nc.scalar.add(out=q_sb[cb][:, ic * ICW:(ic + 1) * ICW], in_=qp,
                              add=bq_t[cb])

    # ================= PHASE 2b: attention =================
    oacc = ctx.enter_context(tc.tile_pool(name=r + "oacc", bufs=1))
    o_acc = [oacc.tile([128, NQ], F32, name=f"{r}oa{cb}", tag=f"oa{cb}") for cb in range(CT)]
    pdenp = ctx.enter_context(tc.tile_pool(name=r + "pden", bufs=1))
    part_den = [pdenp.tile([128, ICW], F32, name=f"{r}pd{ic}", tag=f"pd{ic}")
                for ic in range(ICN)]
    ep = ctx.enter_context(tc.tile_pool(name=r + "ep", bufs=3))
    fin = ctx.enter_context(tc.tile_pool(name=r + "fin", bufs=2))
    op = ctx.enter_context(tc.tile_pool(name=r + "op", bufs=1, space="PSUM"))

    for jh in range(2):
        if jh == 1:
            # reload second halves of K and VT into the same slots
            for cb in range(CT):
                for jc in range(JCN):
                    kn = kpool.tile([128, 512], F32R, name=f"{r}kn{cb}_{jc}",
                                    tag=f"k{cb}_{jc}")
                    nc.sync.dma_start(out=kn, in_=k1d[cb][jc][:].bitcast(F32R))
                    k_sb[cb][jc] = kn
            for jb in range(JBN):
                vn = vpool.tile([128, 512], F32R, name=f"{r}vn{jb}", tag=f"vt{jb}")
                nc.sync.dma_start(out=vn, in_=vt1d[jb][:].bitcast(F32R))
                vt_sb[jb] = vn

        for ic in range(ICN):
            if jh == 1:
                xq_pre = []
                for cb in range(CT):
                    xp = fin.tile([128, ICW], F32, name=f"{r}xp{cb}_{ic}",
                                  tag=f"xp{cb}", bufs=1)
                    nc.sync.dma_start(
                        out=xp,
                        in_=XQ[cb * 128:(cb + 1) * 128, ic * ICW:(ic + 1) * ICW])
                    xq_pre.append(xp)
            o_ps = [op.tile([128, ICW], F32, name=f"{r}o{cb}_{jh}_{ic}", tag=f"o{cb}")
                    for cb in range(CT)]
            for jb in range(JBN):
                st = pps.tile([128, ICW], F32, name=f"{r}s{jh}_{ic}_{jb}", tag="mm")
                for t in range(CT):
                    nc.tensor.matmul(
                        st, k_sb[t][jb // 4][:, (jb % 4) * 128:(jb % 4 + 1) * 128],
                        q_sb[t][:, ic * ICW:(ic + 1) * ICW],
                        start=(t == 0), stop=(t == CT - 1))
                e = ep.tile([128, ICW], F32R, name=f"{r}e{jh}_{ic}_{jb}", tag="e")
                nc.scalar.activation(out=e, in_=st, func=AF.Exp, scale=INV)
                deng = nc.vector if jb % 2 == 0 else nc.gpsimd
                if jh == 0 and jb == 0:
                    deng.tensor_copy(out=part_den[ic], in_=e.bitcast(F32))
                else:
                    deng.tensor_add(out=part_den[ic], in0=part_den[ic],
                                    in1=e.bitcast(F32))
                for cb in range(CT):
                    nc.tensor.matmul(o_ps[cb], vt_sb[jb][:, cb * 128:(cb + 1) * 128],
                                     e, start=(jb == 0), stop=(jb == JBN - 1))

            if jh == 0:
                for cb in range(CT):
                    nc.scalar.copy(out=o_acc[cb][:, ic * ICW:(ic + 1) * ICW],
                                   in_=o_ps[cb])
            else:
                # finalize chunk: o_f, den, proj, residual, store
                o_f = []
                for cb in range(CT):
                    of = ep.tile([128, ICW], F32R, name=f"{r}of{cb}_{ic}", tag=f"of{cb}", bufs=1)
                    nc.vector.tensor_add(
                        out=of, in0=o_ps[cb],
                        in1=o_acc[cb][:, ic * ICW:(ic + 1) * ICW])
                    o_f.append(of)
                da = ep.tile([128, ICW], F32, name=f"{r}da{ic}", tag="da", bufs=2)
                nc.gpsimd.partition_all_reduce(da, part_den[ic],
                                               channels=128,
                                               reduce_op=bass_rust.ReduceOp.add)
                rb = ep.tile([128, ICW], F32, name=f"{r}rb{ic}", tag="rb", bufs=2)
                nc.vector.reciprocal(out=rb, in_=da)
                for cb in range(CT):
                    pp = op.tile([128, ICW], F32, name=f"{r}p{cb}_{ic}", tag=f"o{cb}")
                    for t in range(CT):
                        nc.tensor.matmul(pp, wp_t[t][:, cb * 128:(cb + 1) * 128],
                                         o_f[t], start=(t == 0), stop=(t == CT - 1))
                    xq_c = xq_pre[cb]
                    t1 = fin.tile([128, ICW], F32, name=f"{r}t1{cb}_{ic}", tag="t1")
                    nc.vector.tensor_mul(out=t1, in0=pp, in1=rb)
                    nc.scalar.add(out=t1, in_=t1, add=bpp_t[cb])
                    ot = fin.tile([128, ICW], F32, name=f"{r}ot{cb}_{ic}", tag="ot")
                    nc.vector.tensor_add(out=ot, in0=t1, in1=xq_c)
                    nc.sync.dma_start(
                        out=OUT[cb * 128:(cb + 1) * 128, ic * ICW:(ic + 1) * ICW],
                        in_=ot)


def _build(reps=1):
    from contextlib import ExitStack
    nc = bacc.Bacc()
    tens = {
        "XF": nc.dram_tensor("XF", [C, N], F32, kind="ExternalInput"),
        "XQ": nc.dram_tensor("XQ", [C, NQ], F32, kind="ExternalInput"),
        "WQT": nc.dram_tensor("WQT", [CT, 128, C], F32, kind="ExternalInput"),
        "WKT": nc.dram_tensor("WKT", [CT, 128, C], F32, kind="ExternalInput"),
        "WVT": nc.dram_tensor("WVT", [CT, 128, C], F32, kind="ExternalInput"),
        "WPT": nc.dram_tensor("WPT", [CT, 128, C], F32, kind="ExternalInput"),
        "CVEC": nc.dram_tensor("CVEC", [128, 20], F32, kind="ExternalInput"),
        "GM": nc.dram_tensor("GM", [128, 128], F32, kind="ExternalInput"),
        "OUT": nc.dram_tensor("OUT", [C, NQ], F32, kind="ExternalOutput"),
    }
    with tile.TileContext(nc) as tc:
        from contextlib import ExitStack as ES
        for rep in range(reps):
            with ES() as ctx:
                _emit(nc, tc, ctx, tens, rep)
    nc.finalize()
    return nc


_NC_CACHE = {}


def _get_nc(reps=1):
    if reps not in _NC_CACHE:
        _NC_CACHE[reps] = _build(reps)
    return _NC_CACHE[reps]


def _prep_inputs(x, gn_scale, gn_bias, wq, bq, wk, bk, wv, bv, wp, bp):
    x = np.ascontiguousarray(np.asarray(x, dtype=np.float32))
    B = x.shape[0]
    xb = x.reshape(B, C, N)
    f32 = lambda v: np.ascontiguousarray(np.asarray(v, dtype=np.float32))
    wq, wk, wv, wp = f32(wq), f32(wk), f32(wv), f32(wp)
    bq, bk, bv, bp = f32(bq), f32(bk), f32(bv), f32(bp)
    common = {
        "WQT": f32(wq.T.reshape(CT, 128, C)),
        "WKT": f32(wk.T.reshape(CT, 128, C)),
        "WVT": f32(wv.T.reshape(CT, 128, C)),
        "WPT": f32(wp.T.reshape(CT, 128, C)),
        "CVEC": np.ascontiguousarray(np.concatenate(
            [v.reshape(CT, 128).T for v in
             [bq, bk, (wp @ bv + bp).astype(np.float32),
              f32(gn_scale), f32(gn_bias)]], axis=1), dtype=np.float32),
        "GM": np.kron(np.eye(8, dtype=np.float32),
                      np.full((16, 16), 1.0 / 16.0, np.float32)),
    }
    in_maps = []
    for core in range(8):
        b, h = core // 2, core % 2
        m = dict(common)
        m["XF"] = xb[b]
        m["XQ"] = np.ascontiguousarray(xb[b][:, h * NQ:(h + 1) * NQ])
        in_maps.append(m)
    return in_maps, B


def kernel(**inputs):
    nc = _get_nc(1)
    in_maps, B = _prep_inputs(**inputs)
    res = run_bass_kernel_spmd(nc, in_maps, core_ids=list(range(8)))
    out = np.empty((B, C, N), dtype=np.float32)
    for core in range(8):
        b, h = core // 2, core % 2
        out[b][:, h * NQ:(h + 1) * NQ] = res.results[core]["OUT"]
    return out.reshape(B, C, 64, 64)



# revision 3
# speedup vs baseline: 5.3364x; 5.3364x over previous
"""Trainium2 Bass kernel for AttnBlock — bf16 resident rewrite (v7:
dual-queue phase-1 loads, proj reuses o-banks).

Sharding: 8 cores = 4 batches x 2 query-halves (no collectives).
Each core: GN over its batch, K/V for all 4096 keys (duplicated within
the pair), Q/attention/proj for its 2048 queries.

vs the f32r baseline: all matmul operands bf16 (FWL-eligible weight
loads), K/VT/Q fully SBUF-resident (no DRAM spill/reload), softmax
denominator computed on the PE via a ones-matrix matmul (replaces the
serial DVE/GPSIMD accumulate + partition_all_reduce), h/x released
after phase 1.

Math per core (batch b, query half h, N=4096 keys, NQ=2048 queries):
  h_ = groupnorm(x[b])                    [C, N]   bf16
  K  = WkT.T @ h_ + bk                    [C, N]   bf16
  VT = (h_.T @ WvT)                       [N, C]   bf16 (no bv; folded into bpp)
  Q  = WqT.T @ hq + bq                    [C, NQ]  bf16
  S^T = K.T @ Q * C^-0.5  -> E = exp(S^T) [N, NQ]  bf16 (no max-sub; scores O(5))
  O0 = VT.T @ E (unnorm)  [C, NQ];  den = ones.T @ E  (PE, PSUM-accumulated)
  out = xq + bpp + (WpT.T @ O0) / den     where bpp = Wp@bv + bp
"""
import math
import numpy as np

import concourse.bass as bass
import concourse.bacc as bacc
import concourse.tile as tile
from concourse import mybir
from concourse.bass_utils import run_bass_kernel_spmd

F32 = mybir.dt.float32
BF16 = mybir.dt.bfloat16
AF = mybir.ActivationFunctionType
ALU = mybir.AluOpType

C = 512          # channels
N = 4096         # spatial positions (keys)
NQ = 2048        # queries per core
CT = 4           # channel tiles of 128
ICN = 4          # i-chunks per core
ICW = 512        # i-chunk width
JBN = 32         # j-blocks (128 wide)
GROUPS = 32
EPS = 1e-6
INV = 1.0 / math.sqrt(C)
BN_FMAX = 512


def _emit(nc, tc, ctx, tens, rep):
    r = f"r{rep}_"
    XF, XQ = tens["XF"], tens["XQ"]
    WQT, WKT, WVT, WPT = tens["WQT"], tens["WKT"], tens["WVT"], tens["WPT"]
    GM = tens["GM"]
    OUT = tens["OUT"]

    const = ctx.enter_context(tc.tile_pool(name=r + "const", bufs=1))
    wpool = ctx.enter_context(tc.tile_pool(name=r + "w", bufs=1))
    kpool = ctx.enter_context(tc.tile_pool(name=r + "k", bufs=1))
    vpool = ctx.enter_context(tc.tile_pool(name=r + "v", bufs=1))
    qpool = ctx.enter_context(tc.tile_pool(name=r + "q", bufs=1))
    pps = ctx.enter_context(tc.tile_pool(name=r + "pps", bufs=3, space="PSUM"))

    # ---- persistent tiles
    wq_t = [wpool.tile([128, C], BF16, name=f"{r}wq{t}", tag=f"wq{t}") for t in range(CT)]
    wk_t = [wpool.tile([128, C], BF16, name=f"{r}wk{t}", tag=f"wk{t}") for t in range(CT)]
    wv_t = [wpool.tile([128, C], BF16, name=f"{r}wv{t}", tag=f"wv{t}") for t in range(CT)]
    wp_t = [wpool.tile([128, C], BF16, name=f"{r}wp{t}", tag=f"wp{t}") for t in range(CT)]
    k_sb = [kpool.tile([128, N], BF16, name=f"{r}k{cb}", tag=f"k{cb}") for cb in range(CT)]
    vt_sb = [vpool.tile([128, C], BF16, name=f"{r}vt{jb}", tag=f"vt{jb}")
             for jb in range(JBN)]
    q_sb = [qpool.tile([128, NQ], BF16, name=f"{r}q{cb}", tag=f"q{cb}") for cb in range(CT)]

    # h tiles live from phase 1 until Q/K/V are done
    hp = tc.alloc_tile_pool(name=r + "h", bufs=1)
    h_t = [hp.tile([128, N], BF16, name=f"{r}h{t}", tag=f"h{t}") for t in range(CT)]
    hq_t = [hp.tile([128, NQ], BF16, name=f"{r}hq{t}", tag=f"hq{t}") for t in range(CT)]

    # ================= PHASE 1: load, GN stats, h =================
    gm_t = const.tile([128, 128], F32, name=r + "gm")
    nc.sync.dma_start(out=gm_t, in_=GM[:, :])
    cvec = const.tile([128, 20], F32, name=r + "cvec")
    nc.sync.dma_start(out=cvec, in_=tens["CVEC"][:, :])
    bq_t = [cvec[:, cb:cb + 1] for cb in range(CT)]
    bk_t = [cvec[:, 4 + cb:5 + cb] for cb in range(CT)]
    bpp_t = [cvec[:, 8 + cb:9 + cb] for cb in range(CT)]
    gns_t = [cvec[:, 12 + t:13 + t] for t in range(CT)]
    gnb_t = [cvec[:, 16 + t:17 + t] for t in range(CT)]
    eps_t = const.tile([128, 1], F32, name=r + "eps")
    nc.vector.memset(eps_t, EPS)
    ones_t = const.tile([128, 128], BF16, name=r + "ones")
    nc.vector.memset(ones_t, 1.0)
    a_t = [const.tile([128, 1], F32, name=f"{r}a{t}", tag=f"a{t}") for t in range(CT)]
    c2_t = [const.tile([128, 1], F32, name=f"{r}c2{t}", tag=f"c2{t}") for t in range(CT)]

    with (
        tc.tile_pool(name=r + "ph1", bufs=2) as ph1,
        tc.tile_pool(name=r + "pgn", bufs=2) as pgn,
        tc.tile_pool(name=r + "gps", bufs=2, space="PSUM") as gps,
    ):
        H2 = N // 2
        for t in range(CT):
            dq = nc.sync if t % 2 == 0 else nc.scalar
            xv = ph1.tile([128, N], F32, name=f"{r}x{t}", tag="x")
            for xc in range(8):
                dq.dma_start(
                    out=xv[:, xc * 512:(xc + 1) * 512],
                    in_=XF[t * 128:(t + 1) * 128, xc * 512:(xc + 1) * 512])
            xq = ph1.tile([128, NQ], F32, name=f"{r}xqi{t}", tag="xq")
            for xc in range(2):
                dq.dma_start(
                    out=xq[:, xc * 1024:(xc + 1) * 1024],
                    in_=XQ[t * 128:(t + 1) * 128, xc * 1024:(xc + 1) * 1024])

            # --- per-channel stats -> group stats (via indicator matmul) ---
            stats = pgn.tile([128, N // BN_FMAX, 6], F32, name=f"{r}st{t}", tag="stats")
            for s in range(N // BN_FMAX):
                nc.vector.bn_stats(out=stats[:, s, :], in_=xv[:, s * BN_FMAX:(s + 1) * BN_FMAX])
            mv = pgn.tile([128, 2], F32, name=f"{r}mv{t}", tag="mv")
            nc.vector.bn_aggr(out=mv, in_=stats)
            t2 = pgn.tile([128, 2], F32, name=f"{r}t2{t}", tag="t2")
            nc.vector.tensor_copy(out=t2[:, 0:1], in_=mv[:, 0:1])
            sq = pgn.tile([128, 1], F32, name=f"{r}sq{t}", tag="sq")
            nc.vector.tensor_mul(out=sq, in0=mv[:, 0:1], in1=mv[:, 0:1])
            nc.vector.tensor_add(out=t2[:, 1:2], in0=mv[:, 1:2], in1=sq)
            chp = gps.tile([128, 2], F32, name=f"{r}chp{t}", tag="gp")
            nc.tensor.matmul(chp, gm_t, t2, start=True, stop=True)
            ch = pgn.tile([128, 2], F32, name=f"{r}ch{t}", tag="ch")
            nc.vector.tensor_copy(out=ch, in_=chp)
            gmean, gmsq = ch[:, 0:1], ch[:, 1:2]
            sg = pgn.tile([128, 1], F32, name=f"{r}sg{t}", tag="sg")
            nc.vector.tensor_mul(out=sg, in0=gmean, in1=gmean)
            gv = pgn.tile([128, 1], F32, name=f"{r}gv{t}", tag="gv")
            nc.vector.tensor_sub(out=gv, in0=gmsq, in1=sg)
            nc.scalar.activation(out=gv, in_=gv, func=AF.Sqrt, bias=eps_t, scale=1.0)
            nc.vector.reciprocal(out=gv, in_=gv)
            nc.vector.tensor_mul(out=a_t[t], in0=gv, in1=gns_t[t])
            tmp = pgn.tile([128, 1], F32, name=f"{r}tm{t}", tag="tm")
            nc.vector.tensor_mul(out=tmp, in0=gmean, in1=a_t[t])
            nc.vector.tensor_sub(out=c2_t[t], in0=gnb_t[t], in1=tmp)

            # --- h = x * a + c2 (bf16), hq likewise ---
            for hh in range(2):
                sl = slice(hh * H2, (hh + 1) * H2)
                job = 2 * t + hh
                if job % 8 in (0, 3, 6):
                    nc.scalar.activation(
                        out=h_t[t][:, sl], in_=xv[:, sl],
                        func=AF.Identity, bias=c2_t[t], scale=a_t[t])
                elif job % 8 in (1, 4, 7):
                    nc.vector.tensor_scalar(
                        out=h_t[t][:, sl], in0=xv[:, sl],
                        scalar1=a_t[t], scalar2=c2_t[t], op0=ALU.mult, op1=ALU.add)
                else:
                    nc.gpsimd.tensor_scalar(
                        out=h_t[t][:, sl], in0=xv[:, sl],
                        scalar1=a_t[t], scalar2=c2_t[t], op0=ALU.mult, op1=ALU.add)
            if t % 2 == 0:
                nc.vector.tensor_scalar(out=hq_t[t], in0=xq, scalar1=a_t[t],
                                        scalar2=c2_t[t], op0=ALU.mult, op1=ALU.add)
            else:
                nc.scalar.activation(out=hq_t[t], in_=xq,
                                     func=AF.Identity, bias=c2_t[t], scale=a_t[t])

        for t in range(CT):
            nc.sync.dma_start(out=wv_t[t], in_=WVT[t])
            nc.scalar.dma_start(out=wk_t[t], in_=WKT[t])
        for t in range(CT):
            nc.sync.dma_start(out=wq_t[t], in_=WQT[t])
            nc.scalar.dma_start(out=wp_t[t], in_=WPT[t])

    # ================= PHASE 1b: VT, K, Q =================
    # --- VT = h.T @ WvT : [N, C] ---
    for jb in range(JBN):
        vp = pps.tile([128, C], F32, name=f"{r}vp{jb}", tag="mm")
        for t in range(CT):
            nc.tensor.matmul(vp, h_t[t][:, jb * 128:(jb + 1) * 128], wv_t[t],
                             start=(t == 0), stop=(t == CT - 1))
        if jb % 2 == 0:
            nc.vector.tensor_copy(out=vt_sb[jb], in_=vp)
        else:
            nc.scalar.copy(out=vt_sb[jb], in_=vp)

    # --- K = WkT.T @ h + bk : [C, N] ---
    for cb in range(CT):
        for jc in range(N // 512):
            kp = pps.tile([128, 512], F32, name=f"{r}kp{cb}_{jc}", tag="mm")
            for t in range(CT):
                nc.tensor.matmul(kp, wk_t[t][:, cb * 128:(cb + 1) * 128],
                                 h_t[t][:, jc * 512:(jc + 1) * 512],
                                 start=(t == 0), stop=(t == CT - 1))
            nc.scalar.add(out=k_sb[cb][:, jc * 512:(jc + 1) * 512], in_=kp,
                          add=bk_t[cb])

    # --- Q = WqT.T @ hq + bq : [C, NQ] ---
    for cb in range(CT):
        for ic in range(ICN):
            qp = pps.tile([128, ICW], F32, name=f"{r}qp{cb}_{ic}", tag="mm")
            for t in range(CT):
                nc.tensor.matmul(qp, wq_t[t][:, cb * 128:(cb + 1) * 128],
                                 hq_t[t][:, ic * ICW:(ic + 1) * ICW],
                                 start=(t == 0), stop=(t == CT - 1))
            nc.scalar.add(out=q_sb[cb][:, ic * ICW:(ic + 1) * ICW], in_=qp,
                          add=bq_t[cb])

    hp.release()

    # ================= PHASE 2: attention =================
    ops_ = ctx.enter_context(tc.tile_pool(name=r + "ops", bufs=1, space="PSUM"))
    ep = ctx.enter_context(tc.tile_pool(name=r + "ep", bufs=3))
    fin = ctx.enter_context(tc.tile_pool(name=r + "fin", bufs=2))

    for ic in range(ICN):
        # residual slices (reloaded from DRAM; off critical path) + bpp
        xqb = []
        for cb in range(CT):
            xr = fin.tile([128, ICW], F32, name=f"{r}xr{cb}_{ic}", tag=f"xr{cb}", bufs=1)
            nc.sync.dma_start(
                out=xr, in_=XQ[cb * 128:(cb + 1) * 128, ic * ICW:(ic + 1) * ICW])
            xb = fin.tile([128, ICW], F32, name=f"{r}xb{cb}_{ic}", tag=f"xb{cb}", bufs=1)
            nc.scalar.add(out=xb, in_=xr, add=bpp_t[cb])
            xqb.append(xb)

        o_ps = [ops_.tile([128, ICW], F32, name=f"{r}o{cb}_{ic}", tag=f"o{cb}")
                for cb in range(CT)]
        den_ps = ops_.tile([128, ICW], F32, name=f"{r}dn{ic}", tag="den")
        for jb in range(JBN):
            st = pps.tile([128, ICW], F32, name=f"{r}s{ic}_{jb}", tag="mm")
            for t in range(CT):
                nc.tensor.matmul(
                    st, k_sb[t][:, jb * 128:(jb + 1) * 128],
                    q_sb[t][:, ic * ICW:(ic + 1) * ICW],
                    start=(t == 0), stop=(t == CT - 1))
            e = ep.tile([128, ICW], BF16, name=f"{r}e{ic}_{jb}", tag="e")
            nc.scalar.activation(out=e, in_=st, func=AF.Exp, scale=INV)
            nc.tensor.matmul(den_ps, ones_t, e, start=(jb == 0), stop=(jb == JBN - 1))
            for cb in range(CT):
                nc.tensor.matmul(o_ps[cb], vt_sb[jb][:, cb * 128:(cb + 1) * 128],
                                 e, start=(jb == 0), stop=(jb == JBN - 1))

        rb = fin.tile([128, ICW], F32, name=f"{r}rb{ic}", tag="rb")
        nc.vector.reciprocal(out=rb, in_=den_ps)
        o_f = []
        for cb in range(CT):
            of = ep.tile([128, ICW], BF16, name=f"{r}of{cb}_{ic}", tag=f"of{cb}", bufs=1)
            if cb % 2 == 0:
                nc.scalar.copy(out=of, in_=o_ps[cb])
            else:
                nc.vector.tensor_copy(out=of, in_=o_ps[cb])
            o_f.append(of)
        for cb in range(CT):
            pp = ops_.tile([128, ICW], F32, name=f"{r}p{cb}_{ic}", tag=f"o{cb}")
            for t in range(CT):
                nc.tensor.matmul(pp, wp_t[t][:, cb * 128:(cb + 1) * 128],
                                 o_f[t], start=(t == 0), stop=(t == CT - 1))
            t1 = fin.tile([128, ICW], F32, name=f"{r}t1{cb}_{ic}", tag="t1")
            nc.vector.tensor_mul(out=t1, in0=pp, in1=rb)
            ot = fin.tile([128, ICW], F32, name=f"{r}ot{cb}_{ic}", tag="ot")
            nc.vector.tensor_add(out=ot, in0=t1, in1=xqb[cb])
            nc.sync.dma_start(
                out=OUT[cb * 128:(cb + 1) * 128, ic * ICW:(ic + 1) * ICW],
                in_=ot)


def _build(reps=1):
    from contextlib import ExitStack as ES
    nc = bacc.Bacc()
    tens = {
        "XF": nc.dram_tensor("XF", [C, N], F32, kind="ExternalInput"),
        "XQ": nc.dram_tensor("XQ", [C, NQ], F32, kind="ExternalInput"),
        "WQT": nc.dram_tensor("WQT", [CT, 128, C], BF16, kind="ExternalInput"),
        "WKT": nc.dram_tensor("WKT", [CT, 128, C], BF16, kind="ExternalInput"),
        "WVT": nc.dram_tensor("WVT", [CT, 128, C], BF16, kind="ExternalInput"),
        "WPT": nc.dram_tensor("WPT", [CT, 128, C], BF16, kind="ExternalInput"),
        "CVEC": nc.dram_tensor("CVEC", [128, 20], F32, kind="ExternalInput"),
        "GM": nc.dram_tensor("GM", [128, 128], F32, kind="ExternalInput"),
        "OUT": nc.dram_tensor("OUT", [C, NQ], F32, kind="ExternalOutput"),
    }
    with tile.TileContext(nc) as tc:
        for rep in range(reps):
            with ES() as ctx:
                _emit(nc, tc, ctx, tens, rep)
    nc.finalize()
    return nc


_NC_CACHE = {}


def _get_nc(reps=1):
    if reps not in _NC_CACHE:
        _NC_CACHE[reps] = _build(reps)
    return _NC_CACHE[reps]


def _prep_inputs(x, gn_scale, gn_bias, wq, bq, wk, bk, wv, bv, wp, bp):
    bf16 = mybir.dt.np(BF16)
    x = np.ascontiguousarray(np.asarray(x, dtype=np.float32))
    B = x.shape[0]
    xb = x.reshape(B, C, N)
    f32 = lambda v: np.ascontiguousarray(np.asarray(v, dtype=np.float32))
    wq, wk, wv, wp = f32(wq), f32(wk), f32(wv), f32(wp)
    bq, bk, bv, bp = f32(bq), f32(bk), f32(bv), f32(bp)
    common = {
        "WQT": np.ascontiguousarray(wq.T.reshape(CT, 128, C).astype(bf16)),
        "WKT": np.ascontiguousarray(wk.T.reshape(CT, 128, C).astype(bf16)),
        "WVT": np.ascontiguousarray(wv.T.reshape(CT, 128, C).astype(bf16)),
        "WPT": np.ascontiguousarray(wp.T.reshape(CT, 128, C).astype(bf16)),
        "CVEC": np.ascontiguousarray(np.concatenate(
            [v.reshape(CT, 128).T for v in
             [bq, bk, (wp @ bv + bp).astype(np.float32),
              f32(gn_scale), f32(gn_bias)]], axis=1), dtype=np.float32),
        "GM": np.kron(np.eye(8, dtype=np.float32),
                      np.full((16, 16), 1.0 / 16.0, np.float32)),
    }
    in_maps = []
    for core in range(8):
        b, h = core // 2, core % 2
        m = dict(common)
        m["XF"] = xb[b]
        m["XQ"] = np.ascontiguousarray(xb[b][:, h * NQ:(h + 1) * NQ])
        in_maps.append(m)
    return in_maps, B


def kernel(**inputs):
    nc = _get_nc(1)
    in_maps, B = _prep_inputs(**inputs)
    res = run_bass_kernel_spmd(nc, in_maps, core_ids=list(range(8)))
    out = np.empty((B, C, N), dtype=np.float32)
    for core in range(8):
        b, h = core // 2, core % 2
        out[b][:, h * NQ:(h + 1) * NQ] = res.results[core]["OUT"]
    return out.reshape(B, C, 64, 64)
